# revision 30
# baseline (speedup 1.0000x reference)
"""TRN2 Bass kernel: 16-head MHA (B=2, S=2048, H=1024) sharded over 8 NeuronCores.

Sharding: data-parallel over batch (2) x tensor-parallel over head groups
(4 groups of 4 heads). Each core computes its 4 heads' attention for its batch
and a partial output projection; the host sums the 4 partials per batch,
transposes, and adds the output bias.

Design notes (ScalarE-exp ~147us and PE ~160us-warm are the two big
serial costs; the schedule keeps the PE busy in long contiguous runs,
which also keeps the HAM clock-gate at full rate):
  - QK for a head PAIR is emitted as adjacent matmuls on disjoint PE
    row-tiles (tile_position (0,0) and (64,0) via base_partition) so the
    two 64-contraction matmuls can run CONCURRENTLY in the 128x128 array.
  - The attention loop walks (qb, pair, kc): per slot two QKs, two exps,
    two mask mults.  AV drains strictly head-sequential (one x_ps PSUM
    accumulator, bufs=1) with a LAG: head h's AV runs while head h+1's
    QK/exp executes, so PSUM stays within 8 banks:
    sps 2x[128,1024] (4) + xps 1x[128,1024] (2) + aux 2x[128,512] (2).
  - exp (softmax scale folded in) is the ONLY thing on the Scalar queue
    until the tail.  Mask multiplies go 3:1 DVE:GpSimd (GpSimd Multiply
    runs at 0.42 efficiency ~2.3us/tile vs DVE bf16 2x-mode ~0.7us).
  - Softmax denominators: ones-column trick in vh (PSUM row 64), NR
    reciprocal on DVE, partition broadcast via a tiny fp32 selector
    matmul, then xn = xu * r on DVE.
  - y partials stream out as bf16 (halves output DMA); host combines in
    fp32 and adds the output bias.
"""

import sys

sys.path.insert(0, "/opt/trn_rl_repo")

from contextlib import ExitStack

import numpy as np
import ml_dtypes

import concourse.tile as tile
from concourse import bacc, mybir

BF16 = mybir.dt.bfloat16
F32 = mybir.dt.float32
P = 128

_PROGRAM_CACHE = {}


def build_mha_program(S=2048, HID=1024, NH=4, DK=64, QB=1024, aug=False):
    """Build + compile the per-core SPMD Bass program."""
    D = NH * DK
    assert NH == 4 and DK == 64
    SH = S // P                  # 16 k-chunks
    HT = HID // P                # 8 hidden chunks
    HTa = HT + (1 if aug else 0)
    QBn = S // QB                # 2 q-blocks
    NS = 512                     # matmul free-dim (PSUM bank limit)
    QH = QB // NS                # 2
    DC = D // P                  # 2 head-pair chunks
    NPAIR = NH // 2
    GW = DK + 2                  # 64 data cols + ones col + pad (4B aligned)
    LAG = 16                     # pending p-tiles before AV drain kicks in

    Exp = mybir.ActivationFunctionType.Exp

    nc = bacc.Bacc("TRN2", target_bir_lowering=False, debug=False)

    qT_d = nc.dram_tensor("qT", [HTa * P, S], BF16, kind="ExternalInput").ap()
    kT_d = nc.dram_tensor("kT", [HTa * P, S], BF16, kind="ExternalInput").ap()
    vT_d = nc.dram_tensor("vT", [HTa * P, S], BF16, kind="ExternalInput").ap()
    maskT_d = nc.dram_tensor("maskT", [S, S], BF16, kind="ExternalInput").ap()
    # weights packed [128, HTa*D]: chunk i in columns i*D..(i+1)*D
    wq_d = nc.dram_tensor("wq", [P, HTa * D], BF16, kind="ExternalInput").ap()
    wk_d = nc.dram_tensor("wk", [P, HTa * D], BF16, kind="ExternalInput").ap()
    wv_d = nc.dram_tensor("wv", [P, HTa * D], BF16, kind="ExternalInput").ap()
    wo_d = nc.dram_tensor("wo", [D, HID], BF16, kind="ExternalInput").ap()
    sel_d = nc.dram_tensor("sel", [2, P], F32, kind="ExternalInput").ap()
    y_d = nc.dram_tensor("y", [HID, S], BF16, kind="ExternalOutput").ap()

    with tile.TileContext(nc) as tc:
        with ExitStack() as ctx:
            persist = ctx.enter_context(tc.tile_pool(name="persist", bufs=1))
            qh_t = [persist.tile([P, S], BF16, tag=f"qh{d}", name=f"qh{d}")
                    for d in range(DC)]
            kh_t = [persist.tile([P, S], BF16, tag=f"kh{d}", name=f"kh{d}")
                    for d in range(DC)]
            vh_t = [persist.tile([P, NH * GW], BF16, tag=f"vh{s}",
                                 name=f"vh{s}") for s in range(SH)]
            xu_t = [persist.tile([P, S], BF16, tag=f"xu{p}", name=f"xu{p}")
                    for p in range(NPAIR)]
            wo_t = [persist.tile([P, HID], BF16, tag=f"wo{p}", name=f"wo{p}")
                    for p in range(NPAIR)]
            mask_t = [persist.tile([P, S], BF16, tag=f"m{i}", name=f"m{i}")
                      for i in range(SH)]
            # rowsums/reciprocals in 2 partition rows (hb) so the selector
            # matmul rhs starts at partition 0.  One QB-wide segment,
            # serially reused per (qb, pair).
            rs_t = persist.tile([2, QB], F32, tag="rs", name="rs")
            rr_t = persist.tile([2, QB], F32, tag="rr", name="rr")
            sel2 = persist.tile([2, P], F32, tag="sel2", name="sel2")
            warm = persist.tile([P, 1], F32, tag="warm", name="warm")
            wv_t = persist.tile([P, HTa * D], BF16, tag="wv", name="wv")
            qw_t = persist.tile([P, HTa * D], BF16, tag="qw", name="qw")

            qfp = ctx.enter_context(tc.tile_pool(name="qfp", bufs=1))
            vtp = ctx.enter_context(tc.tile_pool(name="vtp", bufs=1))
            vT_t = [vtp.tile([P, S], BF16, tag=f"vT{i}", name=f"vT{i}")
                    for i in range(HTa)]

            nc.sync.dma_start(sel2[:], sel_d)
            nc.sync.dma_start(wv_t[:], wv_d)
            for pr in range(NPAIR):
                nc.sync.dma_start(wo_t[pr][:], wo_d[pr * P:(pr + 1) * P, :])
            # exp table warm-up so ACT_TABLE_LOAD happens during input DMA
            nc.scalar.activation(warm[:], warm[:], Exp)
            for sc in range(SH):
                nc.vector.memset(vh_t[sc][:], 1.0)

            # ---- q/k projections, streamed by COLUMN blocks so the
            # first exp needs only k-cols 0:512 + q-cols 0:1024.
            with ExitStack() as ph1:
                wqk = ph1.enter_context(tc.tile_pool(name="wqk", bufs=1))
                inr = ph1.enter_context(tc.tile_pool(name="inr", bufs=2))
                pj = ph1.enter_context(
                    tc.tile_pool(name="pj", bufs=2, space="PSUM"))
                wk_t = wqk.tile([P, HTa * D], BF16, tag="wk", name="wk")
                nc.sync.dma_start(qw_t[:], wq_d)
                nc.sync.dma_start(wk_t[:], wk_d)
                blocks = [('k', 0), ('q', 0), ('q', 1), ('k', 1),
                          ('k', 2), ('k', 3)]
                for which, b in blocks:
                    src_d, w_t, dst = ((qT_d, qw_t, qh_t) if which == 'q'
                                       else (kT_d, wk_t, kh_t))
                    csl = slice(b * NS, (b + 1) * NS)
                    t = inr.tile([P, HTa * NS], BF16, tag="inr",
                                 name=f"in{which}{b}")
                    nc.sync.dma_start(
                        t[:].rearrange("p (i c) -> p i c", i=HTa),
                        src_d[:, csl].rearrange("(i p) c -> p i c", p=P))
                    pa = pj.tile([P, NS], F32, tag="pja", name=f"pa{which}{b}")
                    pb = pj.tile([P, NS], F32, tag="pjb", name=f"pb{which}{b}")
                    for i in range(HTa):
                        for dc, pt in ((0, pa), (1, pb)):
                            nc.tensor.matmul(
                                pt[:],
                                w_t[:, i * D + dc * P:i * D + (dc + 1) * P],
                                t[:, i * NS:(i + 1) * NS],
                                start=(i == 0), stop=(i == HTa - 1))
                    nc.vector.tensor_copy(dst[0][:, csl], pa[:])
                    nc.vector.tensor_copy(dst[1][:, csl], pb[:])

            # DMAs consumed mid-attention, in first-use order: mask columns
            # for qb0, then vT (vh filler matmuls), then mask qb1 columns.
            for sc in range(SH):
                nc.sync.dma_start(mask_t[sc][:, 0:QB],
                                  maskT_d[sc * P:(sc + 1) * P, 0:QB])
            for i in range(HTa):
                nc.sync.dma_start(vT_t[i][:], vT_d[i * P:(i + 1) * P, :])
            for sc in range(SH):
                nc.sync.dma_start(mask_t[sc][:, QB:S],
                                  maskT_d[sc * P:(sc + 1) * P, QB:S])

            # ---- attention: pair-interleaved QK->exp->mask, lagged AV ----
            with ExitStack() as ph2:
                pp = ph2.enter_context(tc.tile_pool(name="pexp", bufs=20))
                stg = ph2.enter_context(tc.tile_pool(name="stg", bufs=1))
                ysb = ph2.enter_context(tc.tile_pool(name="ysb", bufs=3))
                sps = ph2.enter_context(
                    tc.tile_pool(name="sps", bufs=2, space="PSUM"))
                xps = ph2.enter_context(
                    tc.tile_pool(name="xps", bufs=1, space="PSUM"))
                aux = ph2.enter_context(
                    tc.tile_pool(name="aux", bufs=2, space="PSUM"))

                x_open = {}

                def emit_av(qb, h, kc, pm_t):
                    last = (qb == QBn - 1 and h == NH - 1)
                    if kc == 0:
                        if last:
                            x_open[h] = (
                                aux.tile([P, NS], F32, tag="aux", name="xta"),
                                aux.tile([P, NS], F32, tag="aux", name="xtb"))
                        else:
                            x_open[h] = xps.tile([P, QB], F32, tag="x",
                                                 name="x")
                    acc = x_open[h]
                    for qh_ in range(QH):
                        nsl = slice(qh_ * NS, (qh_ + 1) * NS)
                        dst = (acc[qh_][:DK + 1, :] if last
                               else acc[:DK + 1, nsl])
                        nc.tensor.matmul(
                            dst,
                            vh_t[kc][:, h * GW:h * GW + DK + 1],
                            pm_t[:, nsl],
                            start=(kc == 0), stop=(kc == SH - 1),
                            skip_group_check=True)
                    if kc == SH - 1:
                        post_head(qb, h, x_open.pop(h))

                def post_head(qb, h, acc):
                    """rowsum out, xu copy, and (on odd heads) normalize."""
                    qsl = slice(qb * QB, (qb + 1) * QB)
                    ht, hb = divmod(h, 2)
                    hsl = slice(64 * hb, 64 * hb + 64)
                    last = (qb == QBn - 1 and h == NH - 1)
                    stage = stg.tile([P, QB], F32, tag="stg", name="stg")
                    if last:
                        for qh_ in range(QH):
                            nsl = slice(qh_ * NS, (qh_ + 1) * NS)
                            nc.vector.tensor_copy(stage[DK:DK + 1, nsl],
                                                  acc[qh_][DK:DK + 1, :])
                            nc.scalar.activation(
                                xu_t[ht][hsl, qb * QB + qh_ * NS:
                                         qb * QB + (qh_ + 1) * NS],
                                acc[qh_][:DK, :],
                                mybir.ActivationFunctionType.Copy)
                    else:
                        nc.vector.tensor_copy(stage[DK:DK + 1, :],
                                              acc[DK:DK + 1, :])
                        nc.vector.tensor_copy(xu_t[ht][hsl, qsl],
                                              acc[:DK, :])
                    nc.sync.dma_start(rs_t[hb:hb + 1, :],
                                      stage[DK:DK + 1, :])
                    if hb == 1:
                        scr = stg.tile([P, QB], F32, tag="stg", name="scr")
                        nc.vector.tensor_scalar_max(rs_t[:], rs_t[:], 1e-30)
                        nc.vector.reciprocal_approx_accurate(
                            rr_t[:], rs_t[:], scr[0:2, :])
                        # qb1-pair0's normalize runs while the last head
                        # holds both aux banks -> its broadcast goes via
                        # the (momentarily free) xps pool instead.
                        via_xps = (qb == QBn - 1 and ht == 0)
                        rb2 = (xps.tile([P, QB], F32, tag="x", name="rbx")
                               if via_xps else None)
                        for qh_ in range(QH):
                            nsl = slice(qh_ * NS, (qh_ + 1) * NS)
                            rb = (rb2[:, nsl] if via_xps else
                                  aux.tile([P, NS], F32, tag="aux",
                                           name="rb")[:])
                            nc.tensor.matmul(
                                rb, sel2[:],
                                rr_t[0:2, nsl],
                                start=True, stop=True,
                                skip_group_check=True)
                            csl = slice(qb * QB + qh_ * NS,
                                        qb * QB + (qh_ + 1) * NS)
                            nc.vector.tensor_mul(
                                xu_t[ht][:, csl], xu_t[ht][:, csl], rb)

                def oproj_chunk(qb, hc):
                    for qh_ in range(QH):
                        qc0 = qb * QH + qh_
                        y_ps = aux.tile([P, NS], F32, tag="aux", name="yps")
                        for pr in range(NPAIR):
                            nc.tensor.matmul(
                                y_ps[:],
                                wo_t[pr][:, hc * P:(hc + 1) * P],
                                xu_t[pr][:, qc0 * NS:(qc0 + 1) * NS],
                                start=(pr == 0), stop=(pr == NPAIR - 1))
                        y_sb = ysb.tile([P, NS], BF16, tag="ysb", name="ysb")
                        if qb == QBn - 1 and (qh_ & 1) == 0:
                            # tail only: ScalarE is idle once exps are done
                            nc.scalar.activation(
                                y_sb[:], y_ps[:],
                                mybir.ActivationFunctionType.Copy)
                        else:
                            nc.vector.tensor_copy(y_sb[:], y_ps[:])
                        nc.sync.dma_start(
                            y_d[hc * P:(hc + 1) * P,
                                qc0 * NS:(qc0 + 1) * NS],
                            y_sb[:])

                # -- PE filler schedule (vh matmuls early, oproj later) --
                # one PSUM accumulator per BANK (first_mm clears the whole
                # bank, so two accumulation groups must not share one).
                fills = []
                vgrp = {}

                def vh_mms(g, i0, i1):
                    def fn():
                        if g not in vgrp:
                            vgrp[g] = (aux.tile([P, NS], F32, tag="aux",
                                                name=f"av{g}a"),
                                       aux.tile([P, NS], F32, tag="aux",
                                                name=f"av{g}b"))
                        ta, tb = vgrp[g]
                        for i in range(i0, i1):
                            for j, t in ((0, ta), (1, tb)):
                                sc = 2 * g + j
                                nc.tensor.matmul(
                                    t[:, 0:D],
                                    vT_t[i][:, sc * P:(sc + 1) * P],
                                    wv_t[:, i * D:(i + 1) * D],
                                    start=(i == 0), stop=(i == HTa - 1))
                    return fn

                def vh_copy(g):
                    def fn():
                        ta, tb = vgrp[g]
                        for j, t in ((0, ta), (1, tb)):
                            sc = 2 * g + j
                            dst = vh_t[sc][:].rearrange(
                                "p (h c) -> p h c", c=GW)[:, :, 0:DK]
                            src = t[:, 0:D].rearrange(
                                "p (h c) -> p h c", c=DK)
                            nc.vector.tensor_copy(dst, src)
                    return fn

                # batch bi needs vT chunks up to 3*bi+2 (gated on DMA
                # arrival); groups serial through the 2-buf aux ring.
                nb = (HTa + 2) // 3
                gate = [2, 4, 6]
                for g in range(SH // 2):
                    ms = 0
                    for bi in range(nb):
                        i0, i1 = 3 * bi, min(3 * bi + 3, HTa)
                        ms = max((2 + 3 * g + bi + 1) // 2, gate[min(bi, 2)])
                        fills.append((ms, vh_mms(g, i0, i1)))
                    fills.append((ms + 1, vh_copy(g)))
                qctx = {}

                def qblk_step(b, i):
                    def fn():
                        if b not in qctx:
                            t = qfp.tile([P, HTa * NS], BF16, tag="qf",
                                         name=f"qf{b}")
                            csl = slice(b * NS, (b + 1) * NS)
                            nc.sync.dma_start(
                                t[:].rearrange("p (i c) -> p i c", i=HTa),
                                qT_d[:, csl].rearrange(
                                    "(i p) c -> p i c", p=P))
                            qctx[b] = (t,
                                       aux.tile([P, NS], F32, tag="aux",
                                                name=f"qa{b}"),
                                       aux.tile([P, NS], F32, tag="aux",
                                                name=f"qb{b}"))
                        t, pa, pb = qctx[b]
                        for dc, pt in ((0, pa), (1, pb)):
                            nc.tensor.matmul(
                                pt[:],
                                qw_t[:, i * D + dc * P:i * D + (dc + 1) * P],
                                t[:, i * NS:(i + 1) * NS],
                                start=(i == 0), stop=(i == HTa - 1))
                    return fn

                def qblk_copy(b):
                    def fn():
                        t, pa, pb = qctx[b]
                        csl = slice(b * NS, (b + 1) * NS)
                        nc.vector.tensor_copy(qh_t[0][:, csl], pa[:])
                        nc.vector.tensor_copy(qh_t[1][:, csl], pb[:])
                    return fn

                for bi_, b in enumerate((2, 3)):
                    base = 8 + bi_ * 13
                    for i in range(HTa):
                        fills.append((base + (3 * i + 1) // 2
                                      if bi_ == 0 else base + i,
                                      qblk_step(b, i)))
                    fills.append((base + ((3 * HTa + 1) // 2
                                          if bi_ == 0 else HTa),
                                  qblk_copy(b)))
                # oproj(qb0) after qb0 fully normalized (~slot 40: qb0's
                # last head drains during qb1-pair0's first half).
                for idx in range(HT):
                    fills.append((40 + idx,
                                  lambda hc=idx: oproj_chunk(0, hc)))
                fills.sort(key=lambda x: x[0])
                fills = fills[::-1]  # pop from end

                # pending AV work: strictly head-sequential drain so only
                # one x_ps accumulator is ever open.
                pend_heads = []   # [[tiles, done_flag], ...] oldest first
                npend = [0]

                def drain_one():
                    """Emit one lagged AV; False if nothing drainable yet
                    (oldest head's tiles all emitted but still producing)."""
                    while pend_heads and not pend_heads[0][0] \
                            and pend_heads[0][1]:
                        pend_heads.pop(0)
                    if pend_heads and pend_heads[0][0]:
                        emit_av(*pend_heads[0][0].pop(0))
                        npend[0] -= 1
                        return True
                    return False

                slot = 0
                mcount = 0
                for qb in range(QBn):
                    qsl = slice(qb * QB, (qb + 1) * QB)
                    for pr in range(NPAIR):
                        final = (qb == QBn - 1 and pr == NPAIR - 1)
                        if final:
                            # one FIFO: E (xps) and O (aux) drain
                            # interleaved so the last head's AV overlaps
                            # the loop instead of running in the tail
                            ent_E = ent_O = [[], False]
                            pend_heads.append(ent_E)
                        else:
                            ent_E, ent_O = [[], False], [[], False]
                            pend_heads.append(ent_E)
                            pend_heads.append(ent_O)
                        for kc in range(SH):
                            s_E = sps.tile([P, QB], F32, tag="s", name="sE")
                            s_O = sps.tile([P, QB], F32, tag="s", name="sO")
                            for qh_ in range(QH):
                                nsl = slice(qh_ * NS, (qh_ + 1) * NS)
                                qcs = slice(qb * QB + qh_ * NS,
                                            qb * QB + (qh_ + 1) * NS)
                                # adjacent disjoint row-tiles -> concurrent
                                nc.tensor.matmul(
                                    s_E[:, nsl],
                                    kh_t[pr][0:64, kc * P:(kc + 1) * P],
                                    qh_t[pr][0:64, qcs],
                                    start=True, stop=True)
                                nc.tensor.matmul(
                                    s_O[:, nsl],
                                    kh_t[pr][64:128, kc * P:(kc + 1) * P],
                                    qh_t[pr][64:128, qcs],
                                    start=True, stop=True)
                            for hb, s_ps in ((0, s_E), (1, s_O)):
                                p_t = pp.tile([P, QB], BF16, tag="p",
                                              name="p")
                                nc.scalar.activation(p_t[:], s_ps[:], Exp,
                                                     scale=0.125)
                                nc.vector.tensor_mul(p_t[:], p_t[:],
                                                     mask_t[kc][:, qsl])
                                mcount += 1
                                ent = ent_E if hb == 0 else ent_O
                                ent[0].append((qb, 2 * pr + hb, kc, p_t))
                                npend[0] += 1
                            if kc == SH - 1:
                                ent_E[1] = ent_O[1] = True
                            lag = (max(4, LAG - 2 * (kc + 1)) if final
                                   else LAG)
                            while npend[0] > lag and drain_one():
                                pass
                            while fills and fills[-1][0] <= slot:
                                fills.pop()[1]()
                            slot += 1
                while npend[0] > 0:
                    if not drain_one():
                        raise RuntimeError("AV drain stuck")
                while fills:
                    fills.pop()[1]()
                for hc in range(HT):
                    oproj_chunk(QBn - 1, hc)

    nc.compile()
    return nc


def make_in_maps(q, k, v, mask, Wq, bq, Wk, bk, Wv, bv, Wo,
                 n_cores=8, NH=4, DK=64, aug=False):
    bf = ml_dtypes.bfloat16
    B, S, HID = q.shape
    D = NH * DK
    n_hg = n_cores // B
    HTa = HID // P + (1 if aug else 0)

    def with_aug(xT, bias_row):
        pad = np.zeros((P, xT.shape[1]), xT.dtype)
        pad[0, :] = bias_row
        return np.concatenate([xT, pad], axis=0)

    def pack_w(w):
        # [HTa*P, D] -> [P, HTa*D] with chunk i in columns i*D..(i+1)*D
        return np.ascontiguousarray(
            w.reshape(HTa, P, D).transpose(1, 0, 2).reshape(P, HTa * D))

    per_batch = {}
    for b in range(B):
        qT = np.ascontiguousarray(q[b].T).astype(bf)
        kT = np.ascontiguousarray(k[b].T).astype(bf)
        vT = np.ascontiguousarray(v[b].T).astype(bf)
        if aug:
            one = np.ones((S,), np.float32).astype(bf)
            qT, kT, vT = with_aug(qT, one), with_aug(kT, one), with_aug(vT, one)
        per_batch[b] = (qT, kT, vT,
                        np.ascontiguousarray(mask[b, 0].T != 0).astype(bf))

    sel = np.zeros((2, P), np.float32)
    sel[0, 0:64] = 1.0
    sel[1, 64:128] = 1.0

    in_maps = []
    for core in range(n_cores):
        b, hg = divmod(core, n_hg)
        hsl = slice(hg * D, (hg + 1) * D)
        wq = Wq[:, hsl].astype(bf)
        wk = Wk[:, hsl].astype(bf)
        wv = Wv[:, hsl].astype(bf)
        if aug:
            wq = with_aug(wq, bq[hsl].astype(bf))
            wk = with_aug(wk, bk[hsl].astype(bf))
            wv = with_aug(wv, bv[hsl].astype(bf))
        qT, kT, vT, mT = per_batch[b]
        in_maps.append(dict(
            qT=qT, kT=kT, vT=vT, maskT=mT,
            wq=pack_w(wq), wk=pack_w(wk), wv=pack_w(wv),
            wo=np.ascontiguousarray(Wo[hsl, :]).astype(bf),
            sel=sel,
        ))
    return in_maps


def combine_outputs(results, B, S, HID, bo, n_cores=8):
    n_hg = n_cores // B
    out = np.zeros((B, S, HID), np.float32)
    for core in range(n_cores):
        b = core // n_hg
        out[b] += results[core]["y"].T.astype(np.float32)
    return out + bo.astype(np.float32)


def run_mha(q, k, v, mask, Wq, bq, Wk, bk, Wv, bv, Wo, bo, trace=False):
    from concourse.bass_utils import run_bass_kernel_spmd

    B, S, HID = q.shape
    n_cores = 8
    aug = bool(np.any(bq) or np.any(bk) or np.any(bv))
    key = (S, HID, aug)
    if key not in _PROGRAM_CACHE:
        _PROGRAM_CACHE[key] = build_mha_program(S=S, HID=HID, aug=aug)
    nc = _PROGRAM_CACHE[key]
    in_maps = make_in_maps(q, k, v, mask, Wq, bq, Wk, bk, Wv, bv, Wo,
                           n_cores=n_cores, aug=aug)
    res = run_bass_kernel_spmd(nc, in_maps, list(range(n_cores)), trace=trace)
    out = combine_outputs(res.results, B, S, HID, bo, n_cores=n_cores)
    return out, res


def kernel(q, k, v, mask, Wq, bq, Wk, bk, Wv, bv, Wo, bo):
    q = np.asarray(q, np.float32)
    k = np.asarray(k, np.float32)
    v = np.asarray(v, np.float32)
    mask = np.asarray(mask)
    out, _ = run_mha(q, k, v, mask,
                     np.asarray(Wq, np.float32), np.asarray(bq, np.float32),
                     np.asarray(Wk, np.float32), np.asarray(bk, np.float32),
                     np.asarray(Wv, np.float32), np.asarray(bv, np.float32),
                     np.asarray(Wo, np.float32), np.asarray(bo, np.float32))
    return out


# revision 31
# speedup vs baseline: 1.1595x; 1.1595x over previous
"""TRN2 Bass kernel: 16-head MHA (B=2, S=2048, H=1024) sharded over 8 NeuronCores.

Sharding: data-parallel over batch (2) x tensor-parallel over head groups
(4 groups of 4 heads). Each core computes its 4 heads' attention for its batch
and a partial output projection; the host sums the 4 partials per batch,
transposes, and adds the output bias.

Design notes (ScalarE-exp ~147us and PE ~160us-warm are the two big
serial costs; the schedule keeps the PE busy in long contiguous runs,
which also keeps the HAM clock-gate at full rate):
  - QK for a head PAIR is emitted as adjacent matmuls on disjoint PE
    row-tiles (tile_position (0,0) and (64,0) via base_partition) so the
    two 64-contraction matmuls can run CONCURRENTLY in the 128x128 array.
  - The attention loop walks (qb, pair, kc): per slot two QKs, two exps,
    two mask mults.  AV drains strictly head-sequential (one x_ps PSUM
    accumulator, bufs=1) with a LAG: head h's AV runs while head h+1's
    QK/exp executes, so PSUM stays within 8 banks:
    sps 2x[128,1024] (4) + xps 1x[128,1024] (2) + aux 2x[128,512] (2).
  - exp (softmax scale folded in) is the ONLY thing on the Scalar queue
    until the tail.  Mask multiplies go 3:1 DVE:GpSimd (GpSimd Multiply
    runs at 0.42 efficiency ~2.3us/tile vs DVE bf16 2x-mode ~0.7us).
  - Softmax denominators: ones-column trick in vh (PSUM row 64), NR
    reciprocal on DVE, partition broadcast via a tiny fp32 selector
    matmul, then xn = xu * r on DVE.
  - y partials stream out as bf16 (halves output DMA); host combines in
    fp32 and adds the output bias.
"""

import sys

sys.path.insert(0, "/opt/trn_rl_repo")

from contextlib import ExitStack

import numpy as np
import ml_dtypes

import concourse.tile as tile
from concourse import bacc, mybir

BF16 = mybir.dt.bfloat16
F32 = mybir.dt.float32
P = 128

_PROGRAM_CACHE = {}


def build_mha_program(S=2048, HID=1024, NH=4, DK=64, QB=1024, aug=False):
    """Build + compile the per-core SPMD Bass program."""
    D = NH * DK
    assert NH == 4 and DK == 64
    SH = S // P                  # 16 k-chunks
    HT = HID // P                # 8 hidden chunks
    HTa = HT + (1 if aug else 0)
    QBn = S // QB                # 2 q-blocks
    NS = 512                     # matmul free-dim (PSUM bank limit)
    QH = QB // NS                # 2
    DC = D // P                  # 2 head-pair chunks
    NPAIR = NH // 2
    GW = DK + 2                  # 64 data cols + ones col + pad (4B aligned)
    LAG = 16                     # pending p-tiles before AV drain kicks in

    Exp = mybir.ActivationFunctionType.Exp

    nc = bacc.Bacc("TRN2", target_bir_lowering=False, debug=False)

    qT_d = nc.dram_tensor("qT", [HTa * P, S], BF16, kind="ExternalInput").ap()
    kT_d = nc.dram_tensor("kT", [HTa * P, S], BF16, kind="ExternalInput").ap()
    vT_d = nc.dram_tensor("vT", [HTa * P, S], BF16, kind="ExternalInput").ap()
    maskT_d = nc.dram_tensor("maskT", [S, S], BF16, kind="ExternalInput").ap()
    # weights packed [128, HTa*D]: chunk i in columns i*D..(i+1)*D
    wq_d = nc.dram_tensor("wq", [P, HTa * D], BF16, kind="ExternalInput").ap()
    wk_d = nc.dram_tensor("wk", [P, HTa * D], BF16, kind="ExternalInput").ap()
    wv_d = nc.dram_tensor("wv", [P, HTa * D], BF16, kind="ExternalInput").ap()
    wo_d = nc.dram_tensor("wo", [D, HID], BF16, kind="ExternalInput").ap()
    sel_d = nc.dram_tensor("sel", [2, P], F32, kind="ExternalInput").ap()
    y_d = nc.dram_tensor("y", [HID, S], BF16, kind="ExternalOutput").ap()

    with tile.TileContext(nc) as tc:
        with ExitStack() as ctx:
            persist = ctx.enter_context(tc.tile_pool(name="persist", bufs=1))
            qh_t = [persist.tile([P, S], BF16, tag=f"qh{d}", name=f"qh{d}")
                    for d in range(DC)]
            kh_t = [persist.tile([P, S], BF16, tag=f"kh{d}", name=f"kh{d}")
                    for d in range(DC)]
            vh_t = [persist.tile([P, NH * GW], BF16, tag=f"vh{s}",
                                 name=f"vh{s}") for s in range(SH)]
            xu_t = [persist.tile([P, S], BF16, tag=f"xu{p}", name=f"xu{p}")
                    for p in range(NPAIR)]
            wo_t = [persist.tile([P, HID], BF16, tag=f"wo{p}", name=f"wo{p}")
                    for p in range(NPAIR)]
            mask_t = [persist.tile([P, S], BF16, tag=f"m{i}", name=f"m{i}")
                      for i in range(SH)]
            # rowsums/reciprocals in 2 partition rows (hb) so the selector
            # matmul rhs starts at partition 0.  One QB-wide segment,
            # serially reused per (qb, pair).
            rs_t = persist.tile([2, QB], F32, tag="rs", name="rs")
            rr_t = persist.tile([2, QB], F32, tag="rr", name="rr")
            sel2 = persist.tile([2, P], F32, tag="sel2", name="sel2")
            warm = persist.tile([P, 1], F32, tag="warm", name="warm")
            wv_t = persist.tile([P, HTa * D], BF16, tag="wv", name="wv")
            qw_t = persist.tile([P, HTa * D], BF16, tag="qw", name="qw")

            qfp = ctx.enter_context(tc.tile_pool(name="qfp", bufs=1))
            vtp = ctx.enter_context(tc.tile_pool(name="vtp", bufs=1))
            vT_t = [vtp.tile([P, S], BF16, tag=f"vT{i}", name=f"vT{i}")
                    for i in range(HTa)]

            nc.sync.dma_start(sel2[:], sel_d)
            nc.sync.dma_start(wv_t[:], wv_d)
            for pr in range(NPAIR):
                nc.sync.dma_start(wo_t[pr][:], wo_d[pr * P:(pr + 1) * P, :])
            # exp table warm-up so ACT_TABLE_LOAD happens during input DMA
            nc.scalar.activation(warm[:], warm[:], Exp)
            for sc in range(SH):
                nc.vector.memset(vh_t[sc][:], 1.0)

            # ---- q/k projections, streamed by COLUMN blocks so the
            # first exp needs only k-cols 0:512 + q-cols 0:1024.
            with ExitStack() as ph1:
                wqk = ph1.enter_context(tc.tile_pool(name="wqk", bufs=1))
                inr = ph1.enter_context(tc.tile_pool(name="inr", bufs=2))
                pj = ph1.enter_context(
                    tc.tile_pool(name="pj", bufs=2, space="PSUM"))
                wk_t = wqk.tile([P, HTa * D], BF16, tag="wk", name="wk")
                nc.sync.dma_start(qw_t[:], wq_d)
                nc.sync.dma_start(wk_t[:], wk_d)
                blocks = [('k', 0), ('q', 0), ('q', 1), ('k', 1),
                          ('k', 2), ('k', 3)]
                for which, b in blocks:
                    src_d, w_t, dst = ((qT_d, qw_t, qh_t) if which == 'q'
                                       else (kT_d, wk_t, kh_t))
                    csl = slice(b * NS, (b + 1) * NS)
                    t = inr.tile([P, HTa * NS], BF16, tag="inr",
                                 name=f"in{which}{b}")
                    nc.sync.dma_start(
                        t[:].rearrange("p (i c) -> p i c", i=HTa),
                        src_d[:, csl].rearrange("(i p) c -> p i c", p=P))
                    pa = pj.tile([P, NS], F32, tag="pja", name=f"pa{which}{b}")
                    pb = pj.tile([P, NS], F32, tag="pjb", name=f"pb{which}{b}")
                    for i in range(HTa):
                        for dc, pt in ((0, pa), (1, pb)):
                            nc.tensor.matmul(
                                pt[:],
                                w_t[:, i * D + dc * P:i * D + (dc + 1) * P],
                                t[:, i * NS:(i + 1) * NS],
                                start=(i == 0), stop=(i == HTa - 1))
                    nc.vector.tensor_copy(dst[0][:, csl], pa[:])
                    nc.vector.tensor_copy(dst[1][:, csl], pb[:])

            # DMAs consumed mid-attention, in first-use order: mask columns
            # for qb0, then vT (vh filler matmuls), then mask qb1 columns.
            for sc in range(SH):
                nc.sync.dma_start(mask_t[sc][:, 0:QB],
                                  maskT_d[sc * P:(sc + 1) * P, 0:QB])
            for i in range(HTa):
                nc.sync.dma_start(vT_t[i][:], vT_d[i * P:(i + 1) * P, :])
            for sc in range(SH):
                nc.sync.dma_start(mask_t[sc][:, QB:S],
                                  maskT_d[sc * P:(sc + 1) * P, QB:S])

            # ---- attention: pair-interleaved QK->exp->mask, lagged AV ----
            with ExitStack() as ph2:
                pp = ph2.enter_context(tc.tile_pool(name="pexp", bufs=20))
                stg = ph2.enter_context(tc.tile_pool(name="stg", bufs=1))
                ysb = ph2.enter_context(tc.tile_pool(name="ysb", bufs=3))
                sps = ph2.enter_context(
                    tc.tile_pool(name="sps", bufs=2, space="PSUM"))
                xps = ph2.enter_context(
                    tc.tile_pool(name="xps", bufs=1, space="PSUM"))
                aux = ph2.enter_context(
                    tc.tile_pool(name="aux", bufs=2, space="PSUM"))

                x_ps_cur = [None]

                def emit_av(qb, h, kc, pm_t):
                    if kc == 0:
                        x_ps_cur[0] = xps.tile([P, QB], F32, tag="x", name="x")
                    x_ps = x_ps_cur[0]
                    for qh_ in range(QH):
                        nsl = slice(qh_ * NS, (qh_ + 1) * NS)
                        nc.tensor.matmul(
                            x_ps[:DK + 1, nsl],
                            vh_t[kc][:, h * GW:h * GW + DK + 1],
                            pm_t[:, nsl],
                            start=(kc == 0), stop=(kc == SH - 1),
                            skip_group_check=True)
                    if kc == SH - 1:
                        post_head(qb, h, x_ps)

                def post_head(qb, h, x_ps):
                    """rowsum out, xu copy, and (on odd heads) normalize."""
                    qsl = slice(qb * QB, (qb + 1) * QB)
                    ht, hb = divmod(h, 2)
                    hsl = slice(64 * hb, 64 * hb + 64)
                    stage = stg.tile([P, QB], F32, tag="stg", name="stg")
                    nc.vector.tensor_copy(stage[DK:DK + 1, :],
                                          x_ps[DK:DK + 1, :])
                    nc.sync.dma_start(rs_t[hb:hb + 1, :],
                                      stage[DK:DK + 1, :])
                    if qb == QBn - 1 and h == NH - 1:
                        nc.scalar.activation(
                            xu_t[ht][hsl, qsl], x_ps[:DK, :],
                            mybir.ActivationFunctionType.Copy)
                    else:
                        nc.vector.tensor_copy(xu_t[ht][hsl, qsl],
                                              x_ps[:DK, :])
                    if hb == 1:
                        scr = stg.tile([P, QB], F32, tag="stg", name="scr")
                        nc.vector.tensor_scalar_max(rs_t[:], rs_t[:], 1e-30)
                        nc.vector.reciprocal_approx_accurate(
                            rr_t[:], rs_t[:], scr[0:2, :])
                        for qh_ in range(QH):
                            rb = aux.tile([P, NS], F32, tag="aux", name="rb")
                            nc.tensor.matmul(
                                rb[:], sel2[:],
                                rr_t[0:2, qh_ * NS:(qh_ + 1) * NS],
                                start=True, stop=True)
                            csl = slice(qb * QB + qh_ * NS,
                                        qb * QB + (qh_ + 1) * NS)
                            nc.vector.tensor_mul(
                                xu_t[ht][:, csl], xu_t[ht][:, csl], rb[:])

                def oproj_chunk(qb, hc):
                    for qh_ in range(QH):
                        qc0 = qb * QH + qh_
                        y_ps = aux.tile([P, NS], F32, tag="aux", name="yps")
                        for pr in range(NPAIR):
                            nc.tensor.matmul(
                                y_ps[:],
                                wo_t[pr][:, hc * P:(hc + 1) * P],
                                xu_t[pr][:, qc0 * NS:(qc0 + 1) * NS],
                                start=(pr == 0), stop=(pr == NPAIR - 1))
                        y_sb = ysb.tile([P, NS], BF16, tag="ysb", name="ysb")
                        if qb == QBn - 1 and (qh_ & 1) == 0:
                            # tail only: ScalarE is idle once exps are done
                            nc.scalar.activation(
                                y_sb[:], y_ps[:],
                                mybir.ActivationFunctionType.Copy)
                        else:
                            nc.vector.tensor_copy(y_sb[:], y_ps[:])
                        nc.sync.dma_start(
                            y_d[hc * P:(hc + 1) * P,
                                qc0 * NS:(qc0 + 1) * NS],
                            y_sb[:])

                # -- PE filler schedule (vh matmuls early, oproj later) --
                # one PSUM accumulator per BANK (first_mm clears the whole
                # bank, so two accumulation groups must not share one).
                fills = []
                vgrp = {}

                def vh_mms(g, i0, i1):
                    def fn():
                        if g not in vgrp:
                            vgrp[g] = (aux.tile([P, NS], F32, tag="aux",
                                                name=f"av{g}a"),
                                       aux.tile([P, NS], F32, tag="aux",
                                                name=f"av{g}b"))
                        ta, tb = vgrp[g]
                        for i in range(i0, i1):
                            for j, t in ((0, ta), (1, tb)):
                                sc = 2 * g + j
                                nc.tensor.matmul(
                                    t[:, 0:D],
                                    vT_t[i][:, sc * P:(sc + 1) * P],
                                    wv_t[:, i * D:(i + 1) * D],
                                    start=(i == 0), stop=(i == HTa - 1))
                    return fn

                def vh_copy(g):
                    def fn():
                        ta, tb = vgrp[g]
                        for j, t in ((0, ta), (1, tb)):
                            sc = 2 * g + j
                            dst = vh_t[sc][:].rearrange(
                                "p (h c) -> p h c", c=GW)[:, :, 0:DK]
                            src = t[:, 0:D].rearrange(
                                "p (h c) -> p h c", c=DK)
                            nc.vector.tensor_copy(dst, src)
                    return fn

                # batch bi needs vT chunks up to 3*bi+2 (gated on DMA
                # arrival); groups serial through the 2-buf aux ring.
                nb = (HTa + 2) // 3
                gate = [2, 4, 6]
                for g in range(SH // 2):
                    ms = 0
                    for bi in range(nb):
                        i0, i1 = 3 * bi, min(3 * bi + 3, HTa)
                        ms = max((2 + 3 * g + bi + 1) // 2, gate[min(bi, 2)])
                        fills.append((ms, vh_mms(g, i0, i1)))
                    fills.append((ms + 1, vh_copy(g)))
                qctx = {}

                def qblk_step(b, i):
                    def fn():
                        if b not in qctx:
                            t = qfp.tile([P, HTa * NS], BF16, tag="qf",
                                         name=f"qf{b}")
                            csl = slice(b * NS, (b + 1) * NS)
                            nc.sync.dma_start(
                                t[:].rearrange("p (i c) -> p i c", i=HTa),
                                qT_d[:, csl].rearrange(
                                    "(i p) c -> p i c", p=P))
                            qctx[b] = (t,
                                       aux.tile([P, NS], F32, tag="aux",
                                                name=f"qa{b}"),
                                       aux.tile([P, NS], F32, tag="aux",
                                                name=f"qb{b}"))
                        t, pa, pb = qctx[b]
                        for dc, pt in ((0, pa), (1, pb)):
                            nc.tensor.matmul(
                                pt[:],
                                qw_t[:, i * D + dc * P:i * D + (dc + 1) * P],
                                t[:, i * NS:(i + 1) * NS],
                                start=(i == 0), stop=(i == HTa - 1))
                    return fn

                def qblk_copy(b):
                    def fn():
                        t, pa, pb = qctx[b]
                        csl = slice(b * NS, (b + 1) * NS)
                        nc.vector.tensor_copy(qh_t[0][:, csl], pa[:])
                        nc.vector.tensor_copy(qh_t[1][:, csl], pb[:])
                    return fn

                for bi_, b in enumerate((2, 3)):
                    base = 8 + bi_ * 13
                    for i in range(HTa):
                        fills.append((base + (3 * i + 1) // 2
                                      if bi_ == 0 else base + i,
                                      qblk_step(b, i)))
                    fills.append((base + ((3 * HTa + 1) // 2
                                          if bi_ == 0 else HTa),
                                  qblk_copy(b)))
                # oproj(qb0) after qb0 fully normalized (~slot 40: qb0's
                # last head drains during qb1-pair0's first half).
                for idx in range(HT):
                    fills.append((42 + idx * 2,
                                  lambda hc=idx: oproj_chunk(0, hc)))
                fills.sort(key=lambda x: x[0])
                fills = fills[::-1]  # pop from end

                # pending AV work: strictly head-sequential drain so only
                # one x_ps accumulator is ever open.
                pend_heads = []   # [[tiles, done_flag], ...] oldest first
                npend = [0]

                def drain_one():
                    """Emit one lagged AV; False if nothing drainable yet
                    (oldest head's tiles all emitted but still producing)."""
                    while pend_heads and not pend_heads[0][0] \
                            and pend_heads[0][1]:
                        pend_heads.pop(0)
                    if pend_heads and pend_heads[0][0]:
                        emit_av(*pend_heads[0][0].pop(0))
                        npend[0] -= 1
                        return True
                    return False

                slot = 0
                mcount = 0
                for qb in range(QBn):
                    qsl = slice(qb * QB, (qb + 1) * QB)
                    for pr in range(NPAIR):
                        ent_E, ent_O = [[], False], [[], False]
                        pend_heads.append(ent_E)
                        pend_heads.append(ent_O)
                        for kc in range(SH):
                            s_E = sps.tile([P, QB], F32, tag="s", name="sE")
                            s_O = sps.tile([P, QB], F32, tag="s", name="sO")
                            for qh_ in range(QH):
                                nsl = slice(qh_ * NS, (qh_ + 1) * NS)
                                qcs = slice(qb * QB + qh_ * NS,
                                            qb * QB + (qh_ + 1) * NS)
                                # adjacent disjoint row-tiles -> concurrent
                                nc.tensor.matmul(
                                    s_E[:, nsl],
                                    kh_t[pr][0:64, kc * P:(kc + 1) * P],
                                    qh_t[pr][0:64, qcs],
                                    start=True, stop=True)
                                nc.tensor.matmul(
                                    s_O[:, nsl],
                                    kh_t[pr][64:128, kc * P:(kc + 1) * P],
                                    qh_t[pr][64:128, qcs],
                                    start=True, stop=True)
                            for hb, s_ps in ((0, s_E), (1, s_O)):
                                p_t = pp.tile([P, QB], BF16, tag="p",
                                              name="p")
                                nc.scalar.activation(p_t[:], s_ps[:], Exp,
                                                     scale=0.125)
                                nc.vector.tensor_mul(p_t[:], p_t[:],
                                                     mask_t[kc][:, qsl])
                                mcount += 1
                                ent = ent_E if hb == 0 else ent_O
                                ent[0].append((qb, 2 * pr + hb, kc, p_t))
                                npend[0] += 1
                            if kc == SH - 1:
                                ent_E[1] = ent_O[1] = True
                            while npend[0] > LAG and drain_one():
                                pass
                            while fills and fills[-1][0] <= slot:
                                fills.pop()[1]()
                            slot += 1
                while npend[0] > 0:
                    if not drain_one():
                        raise RuntimeError("AV drain stuck")
                while fills:
                    fills.pop()[1]()
                for hc in range(HT):
                    oproj_chunk(QBn - 1, hc)

    nc.compile()
    return nc


def make_in_maps(q, k, v, mask, Wq, bq, Wk, bk, Wv, bv, Wo,
                 n_cores=8, NH=4, DK=64, aug=False):
    bf = ml_dtypes.bfloat16
    B, S, HID = q.shape
    D = NH * DK
    n_hg = n_cores // B
    HTa = HID // P + (1 if aug else 0)

    def with_aug(xT, bias_row):
        pad = np.zeros((P, xT.shape[1]), xT.dtype)
        pad[0, :] = bias_row
        return np.concatenate([xT, pad], axis=0)

    def pack_w(w):
        # [HTa*P, D] -> [P, HTa*D] with chunk i in columns i*D..(i+1)*D
        return np.ascontiguousarray(
            w.reshape(HTa, P, D).transpose(1, 0, 2).reshape(P, HTa * D))

    per_batch = {}
    for b in range(B):
        qT = np.ascontiguousarray(q[b].T).astype(bf)
        kT = np.ascontiguousarray(k[b].T).astype(bf)
        vT = np.ascontiguousarray(v[b].T).astype(bf)
        if aug:
            one = np.ones((S,), np.float32).astype(bf)
            qT, kT, vT = with_aug(qT, one), with_aug(kT, one), with_aug(vT, one)
        per_batch[b] = (qT, kT, vT,
                        np.ascontiguousarray(mask[b, 0].T != 0).astype(bf))

    sel = np.zeros((2, P), np.float32)
    sel[0, 0:64] = 1.0
    sel[1, 64:128] = 1.0

    in_maps = []
    for core in range(n_cores):
        b, hg = divmod(core, n_hg)
        hsl = slice(hg * D, (hg + 1) * D)
        wq = Wq[:, hsl].astype(bf)
        wk = Wk[:, hsl].astype(bf)
        wv = Wv[:, hsl].astype(bf)
        if aug:
            wq = with_aug(wq, bq[hsl].astype(bf))
            wk = with_aug(wk, bk[hsl].astype(bf))
            wv = with_aug(wv, bv[hsl].astype(bf))
        qT, kT, vT, mT = per_batch[b]
        in_maps.append(dict(
            qT=qT, kT=kT, vT=vT, maskT=mT,
            wq=pack_w(wq), wk=pack_w(wk), wv=pack_w(wv),
            wo=np.ascontiguousarray(Wo[hsl, :]).astype(bf),
            sel=sel,
        ))
    return in_maps


def combine_outputs(results, B, S, HID, bo, n_cores=8):
    n_hg = n_cores // B
    out = np.zeros((B, S, HID), np.float32)
    for core in range(n_cores):
        b = core // n_hg
        out[b] += results[core]["y"].T.astype(np.float32)
    return out + bo.astype(np.float32)


def run_mha(q, k, v, mask, Wq, bq, Wk, bk, Wv, bv, Wo, bo, trace=False):
    from concourse.bass_utils import run_bass_kernel_spmd

    B, S, HID = q.shape
    n_cores = 8
    aug = bool(np.any(bq) or np.any(bk) or np.any(bv))
    key = (S, HID, aug)
    if key not in _PROGRAM_CACHE:
        _PROGRAM_CACHE[key] = build_mha_program(S=S, HID=HID, aug=aug)
    nc = _PROGRAM_CACHE[key]
    in_maps = make_in_maps(q, k, v, mask, Wq, bq, Wk, bk, Wv, bv, Wo,
                           n_cores=n_cores, aug=aug)
    res = run_bass_kernel_spmd(nc, in_maps, list(range(n_cores)), trace=trace)
    out = combine_outputs(res.results, B, S, HID, bo, n_cores=n_cores)
    return out, res


def kernel(q, k, v, mask, Wq, bq, Wk, bk, Wv, bv, Wo, bo):
    q = np.asarray(q, np.float32)
    k = np.asarray(k, np.float32)
    v = np.asarray(v, np.float32)
    mask = np.asarray(mask)
    out, _ = run_mha(q, k, v, mask,
                     np.asarray(Wq, np.float32), np.asarray(bq, np.float32),
                     np.asarray(Wk, np.float32), np.asarray(bk, np.float32),
                     np.asarray(Wv, np.float32), np.asarray(bv, np.float32),
                     np.asarray(Wo, np.float32), np.asarray(bo, np.float32))
    return out


# revision 32
# speedup vs baseline: 1.1908x; 1.0269x over previous
"""TRN2 Bass kernel: 16-head MHA (B=2, S=2048, H=1024) sharded over 8 NeuronCores.

Sharding: data-parallel over batch (2) x tensor-parallel over head groups
(4 groups of 4 heads). Each core computes its 4 heads' attention for its batch
and a partial output projection; the host sums the 4 partials per batch,
transposes, and adds the output bias.

Design notes (ScalarE-exp ~147us and PE ~160us-warm are the two big
serial costs; the schedule keeps the PE busy in long contiguous runs,
which also keeps the HAM clock-gate at full rate):
  - QK for a head PAIR is emitted as adjacent matmuls on disjoint PE
    row-tiles (tile_position (0,0) and (64,0) via base_partition) so the
    two 64-contraction matmuls can run CONCURRENTLY in the 128x128 array.
  - The attention loop walks (qb, pair, kc): per slot two QKs, two exps,
    two mask mults.  AV drains strictly head-sequential (one x_ps PSUM
    accumulator, bufs=1) with a LAG: head h's AV runs while head h+1's
    QK/exp executes, so PSUM stays within 8 banks:
    sps 2x[128,1024] (4) + xps 1x[128,1024] (2) + aux 2x[128,512] (2).
  - exp (softmax scale folded in) is the ONLY thing on the Scalar queue
    until the tail.  Mask multiplies go 3:1 DVE:GpSimd (GpSimd Multiply
    runs at 0.42 efficiency ~2.3us/tile vs DVE bf16 2x-mode ~0.7us).
  - Softmax denominators: ones-column trick in vh (PSUM row 64), NR
    reciprocal on DVE, partition broadcast via a tiny fp32 selector
    matmul, then xn = xu * r on DVE.
  - y partials stream out as bf16 (halves output DMA); host combines in
    fp32 and adds the output bias.
"""

import sys

sys.path.insert(0, "/opt/trn_rl_repo")

from contextlib import ExitStack

import numpy as np
import ml_dtypes

import concourse.tile as tile
from concourse import bacc, mybir

BF16 = mybir.dt.bfloat16
F32 = mybir.dt.float32
P = 128

_PROGRAM_CACHE = {}


def build_mha_program(S=2048, HID=1024, NH=4, DK=64, QB=1024, aug=False):
    """Build + compile the per-core SPMD Bass program."""
    D = NH * DK
    assert NH == 4 and DK == 64
    SH = S // P                  # 16 k-chunks
    HT = HID // P                # 8 hidden chunks
    HTa = HT + (1 if aug else 0)
    QBn = S // QB                # 2 q-blocks
    NS = 512                     # matmul free-dim (PSUM bank limit)
    QH = QB // NS                # 2
    DC = D // P                  # 2 head-pair chunks
    NPAIR = NH // 2
    GW = DK + 2                  # 64 data cols + ones col + pad (4B aligned)
    LAG = 16                     # pending p-tiles before AV drain kicks in

    Exp = mybir.ActivationFunctionType.Exp

    nc = bacc.Bacc("TRN2", target_bir_lowering=False, debug=False)

    qT_d = nc.dram_tensor("qT", [HTa * P, S], BF16, kind="ExternalInput").ap()
    kT_d = nc.dram_tensor("kT", [HTa * P, S], BF16, kind="ExternalInput").ap()
    vT_d = nc.dram_tensor("vT", [HTa * P, S], BF16, kind="ExternalInput").ap()
    maskT_d = nc.dram_tensor("maskT", [S, S], BF16, kind="ExternalInput").ap()
    # weights packed [128, HTa*D]: chunk i in columns i*D..(i+1)*D
    wq_d = nc.dram_tensor("wq", [P, HTa * D], BF16, kind="ExternalInput").ap()
    wk_d = nc.dram_tensor("wk", [P, HTa * D], BF16, kind="ExternalInput").ap()
    wv_d = nc.dram_tensor("wv", [P, HTa * D], BF16, kind="ExternalInput").ap()
    wo_d = nc.dram_tensor("wo", [D, HID], BF16, kind="ExternalInput").ap()
    sel_d = nc.dram_tensor("sel", [2, P], F32, kind="ExternalInput").ap()
    y_d = nc.dram_tensor("y", [HID, S], BF16, kind="ExternalOutput").ap()

    with tile.TileContext(nc) as tc:
        with ExitStack() as ctx:
            persist = ctx.enter_context(tc.tile_pool(name="persist", bufs=1))
            qh_t = [persist.tile([P, S], BF16, tag=f"qh{d}", name=f"qh{d}")
                    for d in range(DC)]
            kh_t = [persist.tile([P, S], BF16, tag=f"kh{d}", name=f"kh{d}")
                    for d in range(DC)]
            vh_t = [persist.tile([P, NH * GW], BF16, tag=f"vh{s}",
                                 name=f"vh{s}") for s in range(SH)]
            xu_t = [persist.tile([P, S], BF16, tag=f"xu{p}", name=f"xu{p}")
                    for p in range(NPAIR)]
            wo_t = [persist.tile([P, HID], BF16, tag=f"wo{p}", name=f"wo{p}")
                    for p in range(NPAIR)]
            mask_t = [persist.tile([P, S], BF16, tag=f"m{i}", name=f"m{i}")
                      for i in range(SH)]
            # rowsums/reciprocals in 2 partition rows (hb) so the selector
            # matmul rhs starts at partition 0.  One QB-wide segment,
            # serially reused per (qb, pair).
            rs_t = persist.tile([2, QB], F32, tag="rs", name="rs")
            rr_t = persist.tile([2, QB], F32, tag="rr", name="rr")
            sel2 = persist.tile([2, P], F32, tag="sel2", name="sel2")
            warm = persist.tile([P, 1], F32, tag="warm", name="warm")
            wv_t = persist.tile([P, HTa * D], BF16, tag="wv", name="wv")
            qw_t = persist.tile([P, HTa * D], BF16, tag="qw", name="qw")

            qfp = ctx.enter_context(tc.tile_pool(name="qfp", bufs=1))
            vtp = ctx.enter_context(tc.tile_pool(name="vtp", bufs=1))
            vT_t = [vtp.tile([P, S], BF16, tag=f"vT{i}", name=f"vT{i}")
                    for i in range(HTa)]

            nc.sync.dma_start(sel2[:], sel_d)
            nc.sync.dma_start(wv_t[:], wv_d)
            for pr in range(NPAIR):
                nc.sync.dma_start(wo_t[pr][:], wo_d[pr * P:(pr + 1) * P, :])
            # exp table warm-up so ACT_TABLE_LOAD happens during input DMA
            nc.scalar.activation(warm[:], warm[:], Exp)
            for sc in range(SH):
                nc.vector.memset(vh_t[sc][:], 1.0)

            # ---- q/k projections, streamed by COLUMN blocks so the
            # first exp needs only k-cols 0:512 + q-cols 0:1024.
            with ExitStack() as ph1:
                wqk = ph1.enter_context(tc.tile_pool(name="wqk", bufs=1))
                inr = ph1.enter_context(tc.tile_pool(name="inr", bufs=2))
                pj = ph1.enter_context(
                    tc.tile_pool(name="pj", bufs=2, space="PSUM"))
                wk_t = wqk.tile([P, HTa * D], BF16, tag="wk", name="wk")
                nc.sync.dma_start(qw_t[:], wq_d)
                nc.sync.dma_start(wk_t[:], wk_d)
                blocks = [('k', 0), ('q', 0), ('q', 1), ('k', 1),
                          ('k', 2), ('k', 3)]
                for which, b in blocks:
                    src_d, w_t, dst = ((qT_d, qw_t, qh_t) if which == 'q'
                                       else (kT_d, wk_t, kh_t))
                    csl = slice(b * NS, (b + 1) * NS)
                    t = inr.tile([P, HTa * NS], BF16, tag="inr",
                                 name=f"in{which}{b}")
                    nc.sync.dma_start(
                        t[:].rearrange("p (i c) -> p i c", i=HTa),
                        src_d[:, csl].rearrange("(i p) c -> p i c", p=P))
                    pa = pj.tile([P, NS], F32, tag="pja", name=f"pa{which}{b}")
                    pb = pj.tile([P, NS], F32, tag="pjb", name=f"pb{which}{b}")
                    for i in range(HTa):
                        for dc, pt in ((0, pa), (1, pb)):
                            nc.tensor.matmul(
                                pt[:],
                                w_t[:, i * D + dc * P:i * D + (dc + 1) * P],
                                t[:, i * NS:(i + 1) * NS],
                                start=(i == 0), stop=(i == HTa - 1))
                    nc.vector.tensor_copy(dst[0][:, csl], pa[:])
                    nc.vector.tensor_copy(dst[1][:, csl], pb[:])

            # DMAs consumed mid-attention, in first-use order: mask columns
            # for qb0, then vT (vh filler matmuls), then mask qb1 columns.
            for sc in range(SH):
                nc.sync.dma_start(mask_t[sc][:, 0:QB],
                                  maskT_d[sc * P:(sc + 1) * P, 0:QB])
            for i in range(HTa):
                nc.sync.dma_start(vT_t[i][:], vT_d[i * P:(i + 1) * P, :])
            for sc in range(SH):
                nc.sync.dma_start(mask_t[sc][:, QB:S],
                                  maskT_d[sc * P:(sc + 1) * P, QB:S])

            # ---- attention: pair-interleaved QK->exp->mask, lagged AV ----
            with ExitStack() as ph2:
                pp = ph2.enter_context(tc.tile_pool(name="pexp", bufs=20))
                stg = ph2.enter_context(tc.tile_pool(name="stg", bufs=1))
                ysb = ph2.enter_context(tc.tile_pool(name="ysb", bufs=3))
                sps = ph2.enter_context(
                    tc.tile_pool(name="sps", bufs=2, space="PSUM"))
                xps = ph2.enter_context(
                    tc.tile_pool(name="xps", bufs=1, space="PSUM"))
                aux = ph2.enter_context(
                    tc.tile_pool(name="aux", bufs=2, space="PSUM"))

                x_open = {}

                def emit_av(qb, h, kc, pm_t):
                    last = (qb == QBn - 1 and h == NH - 1)
                    if kc == 0:
                        if last:
                            x_open[h] = (
                                aux.tile([P, NS], F32, tag="aux", name="xta"),
                                aux.tile([P, NS], F32, tag="aux", name="xtb"))
                        else:
                            x_open[h] = xps.tile([P, QB], F32, tag="x",
                                                 name="x")
                    acc = x_open[h]
                    for qh_ in range(QH):
                        nsl = slice(qh_ * NS, (qh_ + 1) * NS)
                        dst = (acc[qh_][:DK + 1, :] if last
                               else acc[:DK + 1, nsl])
                        nc.tensor.matmul(
                            dst,
                            vh_t[kc][:, h * GW:h * GW + DK + 1],
                            pm_t[:, nsl],
                            start=(kc == 0), stop=(kc == SH - 1),
                            skip_group_check=True)
                    if kc == SH - 1:
                        post_head(qb, h, x_open.pop(h))

                def post_head(qb, h, acc):
                    """rowsum out, xu copy, and (on odd heads) normalize."""
                    qsl = slice(qb * QB, (qb + 1) * QB)
                    ht, hb = divmod(h, 2)
                    hsl = slice(64 * hb, 64 * hb + 64)
                    last = (qb == QBn - 1 and h == NH - 1)
                    stage = stg.tile([P, QB], F32, tag="stg", name="stg")
                    if last:
                        for qh_ in range(QH):
                            nsl = slice(qh_ * NS, (qh_ + 1) * NS)
                            nc.vector.tensor_copy(stage[DK:DK + 1, nsl],
                                                  acc[qh_][DK:DK + 1, :])
                            nc.scalar.activation(
                                xu_t[ht][hsl, qb * QB + qh_ * NS:
                                         qb * QB + (qh_ + 1) * NS],
                                acc[qh_][:DK, :],
                                mybir.ActivationFunctionType.Copy)
                    else:
                        nc.vector.tensor_copy(stage[DK:DK + 1, :],
                                              acc[DK:DK + 1, :])
                        nc.vector.tensor_copy(xu_t[ht][hsl, qsl],
                                              acc[:DK, :])
                    nc.sync.dma_start(rs_t[hb:hb + 1, :],
                                      stage[DK:DK + 1, :])
                    if hb == 1:
                        scr = stg.tile([P, QB], F32, tag="stg", name="scr")
                        nc.vector.tensor_scalar_max(rs_t[:], rs_t[:], 1e-30)
                        nc.vector.reciprocal_approx_accurate(
                            rr_t[:], rs_t[:], scr[0:2, :])
                        via_xps = (qb == QBn - 1 and ht == 0)
                        rb2 = (xps.tile([P, QB], F32, tag="x", name="rbx")
                               if via_xps else None)
                        for qh_ in range(QH):
                            nsl = slice(qh_ * NS, (qh_ + 1) * NS)
                            rb = (rb2[:, nsl] if via_xps else
                                  aux.tile([P, NS], F32, tag="aux",
                                           name="rb")[:])
                            nc.tensor.matmul(
                                rb, sel2[:],
                                rr_t[0:2, nsl],
                                start=True, stop=True,
                                skip_group_check=True)
                            csl = slice(qb * QB + qh_ * NS,
                                        qb * QB + (qh_ + 1) * NS)
                            nc.vector.tensor_mul(
                                xu_t[ht][:, csl], xu_t[ht][:, csl], rb)

                def oproj_chunk(qb, hc):
                    for qh_ in range(QH):
                        qc0 = qb * QH + qh_
                        y_ps = aux.tile([P, NS], F32, tag="aux", name="yps")
                        for pr in range(NPAIR):
                            nc.tensor.matmul(
                                y_ps[:],
                                wo_t[pr][:, hc * P:(hc + 1) * P],
                                xu_t[pr][:, qc0 * NS:(qc0 + 1) * NS],
                                start=(pr == 0), stop=(pr == NPAIR - 1))
                        y_sb = ysb.tile([P, NS], BF16, tag="ysb", name="ysb")
                        if qb == QBn - 1 and (qh_ & 1) == 0:
                            # tail only: ScalarE is idle once exps are done
                            nc.scalar.activation(
                                y_sb[:], y_ps[:],
                                mybir.ActivationFunctionType.Copy)
                        else:
                            nc.vector.tensor_copy(y_sb[:], y_ps[:])
                        nc.sync.dma_start(
                            y_d[hc * P:(hc + 1) * P,
                                qc0 * NS:(qc0 + 1) * NS],
                            y_sb[:])

                # -- PE filler schedule (vh matmuls early, oproj later) --
                # one PSUM accumulator per BANK (first_mm clears the whole
                # bank, so two accumulation groups must not share one).
                fills = []
                vgrp = {}

                def vh_mms(g, i0, i1):
                    def fn():
                        if g not in vgrp:
                            vgrp[g] = (aux.tile([P, NS], F32, tag="aux",
                                                name=f"av{g}a"),
                                       aux.tile([P, NS], F32, tag="aux",
                                                name=f"av{g}b"))
                        ta, tb = vgrp[g]
                        for i in range(i0, i1):
                            for j, t in ((0, ta), (1, tb)):
                                sc = 2 * g + j
                                nc.tensor.matmul(
                                    t[:, 0:D],
                                    vT_t[i][:, sc * P:(sc + 1) * P],
                                    wv_t[:, i * D:(i + 1) * D],
                                    start=(i == 0), stop=(i == HTa - 1))
                    return fn

                def vh_copy(g):
                    def fn():
                        ta, tb = vgrp[g]
                        for j, t in ((0, ta), (1, tb)):
                            sc = 2 * g + j
                            dst = vh_t[sc][:].rearrange(
                                "p (h c) -> p h c", c=GW)[:, :, 0:DK]
                            src = t[:, 0:D].rearrange(
                                "p (h c) -> p h c", c=DK)
                            nc.vector.tensor_copy(dst, src)
                    return fn

                # batch bi needs vT chunks up to 3*bi+2 (gated on DMA
                # arrival); groups serial through the 2-buf aux ring.
                nb = (HTa + 2) // 3
                gate = [2, 4, 6]
                for g in range(SH // 2):
                    ms = 0
                    for bi in range(nb):
                        i0, i1 = 3 * bi, min(3 * bi + 3, HTa)
                        ms = max((2 + 3 * g + bi + 1) // 2, gate[min(bi, 2)])
                        fills.append((ms, vh_mms(g, i0, i1)))
                    fills.append((ms + 1, vh_copy(g)))
                qctx = {}

                def qblk_step(b, i):
                    def fn():
                        if b not in qctx:
                            t = qfp.tile([P, HTa * NS], BF16, tag="qf",
                                         name=f"qf{b}")
                            csl = slice(b * NS, (b + 1) * NS)
                            nc.sync.dma_start(
                                t[:].rearrange("p (i c) -> p i c", i=HTa),
                                qT_d[:, csl].rearrange(
                                    "(i p) c -> p i c", p=P))
                            qctx[b] = (t,
                                       aux.tile([P, NS], F32, tag="aux",
                                                name=f"qa{b}"),
                                       aux.tile([P, NS], F32, tag="aux",
                                                name=f"qb{b}"))
                        t, pa, pb = qctx[b]
                        for dc, pt in ((0, pa), (1, pb)):
                            nc.tensor.matmul(
                                pt[:],
                                qw_t[:, i * D + dc * P:i * D + (dc + 1) * P],
                                t[:, i * NS:(i + 1) * NS],
                                start=(i == 0), stop=(i == HTa - 1))
                    return fn

                def qblk_copy(b):
                    def fn():
                        t, pa, pb = qctx[b]
                        csl = slice(b * NS, (b + 1) * NS)
                        nc.vector.tensor_copy(qh_t[0][:, csl], pa[:])
                        nc.vector.tensor_copy(qh_t[1][:, csl], pb[:])
                    return fn

                for bi_, b in enumerate((2, 3)):
                    base = 8 + bi_ * 13
                    for i in range(HTa):
                        fills.append((base + (3 * i + 1) // 2
                                      if bi_ == 0 else base + i,
                                      qblk_step(b, i)))
                    fills.append((base + ((3 * HTa + 1) // 2
                                          if bi_ == 0 else HTa),
                                  qblk_copy(b)))
                # oproj(qb0) after qb0 fully normalized (~slot 40: qb0's
                # last head drains during qb1-pair0's first half).
                for idx in range(HT):
                    fills.append((42 + idx * 2,
                                  lambda hc=idx: oproj_chunk(0, hc)))
                fills.sort(key=lambda x: x[0])
                fills = fills[::-1]  # pop from end

                # pending AV work: strictly head-sequential drain so only
                # one x_ps accumulator is ever open.
                pend_heads = []   # [[tiles, done_flag], ...] oldest first
                npend = [0]

                def drain_one():
                    """Emit one lagged AV; False if nothing drainable yet
                    (oldest head's tiles all emitted but still producing)."""
                    while pend_heads and not pend_heads[0][0] \
                            and pend_heads[0][1]:
                        pend_heads.pop(0)
                    if pend_heads and pend_heads[0][0]:
                        emit_av(*pend_heads[0][0].pop(0))
                        npend[0] -= 1
                        return True
                    return False

                slot = 0
                mcount = 0
                for qb in range(QBn):
                    qsl = slice(qb * QB, (qb + 1) * QB)
                    for pr in range(NPAIR):
                        final = (qb == QBn - 1 and pr == NPAIR - 1)
                        if final:
                            ent_E = ent_O = [[], False]
                            pend_heads.append(ent_E)
                        else:
                            ent_E, ent_O = [[], False], [[], False]
                            pend_heads.append(ent_E)
                            pend_heads.append(ent_O)
                        for kc in range(SH):
                            s_E = sps.tile([P, QB], F32, tag="s", name="sE")
                            s_O = sps.tile([P, QB], F32, tag="s", name="sO")
                            for qh_ in range(QH):
                                nsl = slice(qh_ * NS, (qh_ + 1) * NS)
                                qcs = slice(qb * QB + qh_ * NS,
                                            qb * QB + (qh_ + 1) * NS)
                                # adjacent disjoint row-tiles -> concurrent
                                nc.tensor.matmul(
                                    s_E[:, nsl],
                                    kh_t[pr][0:64, kc * P:(kc + 1) * P],
                                    qh_t[pr][0:64, qcs],
                                    start=True, stop=True)
                                nc.tensor.matmul(
                                    s_O[:, nsl],
                                    kh_t[pr][64:128, kc * P:(kc + 1) * P],
                                    qh_t[pr][64:128, qcs],
                                    start=True, stop=True)
                            for hb, s_ps in ((0, s_E), (1, s_O)):
                                p_t = pp.tile([P, QB], BF16, tag="p",
                                              name="p")
                                nc.scalar.activation(p_t[:], s_ps[:], Exp,
                                                     scale=0.125)
                                nc.vector.tensor_mul(p_t[:], p_t[:],
                                                     mask_t[kc][:, qsl])
                                mcount += 1
                                ent = ent_E if hb == 0 else ent_O
                                ent[0].append((qb, 2 * pr + hb, kc, p_t))
                                npend[0] += 1
                            if kc == SH - 1:
                                ent_E[1] = ent_O[1] = True
                            lag = (max(6, LAG - (kc + 1)) if final
                                   else LAG)
                            while npend[0] > lag and drain_one():
                                pass
                            while fills and fills[-1][0] <= slot:
                                fills.pop()[1]()
                            slot += 1
                while npend[0] > 0:
                    if not drain_one():
                        raise RuntimeError("AV drain stuck")
                while fills:
                    fills.pop()[1]()
                for hc in range(HT):
                    oproj_chunk(QBn - 1, hc)

    nc.compile()
    return nc


def make_in_maps(q, k, v, mask, Wq, bq, Wk, bk, Wv, bv, Wo,
                 n_cores=8, NH=4, DK=64, aug=False):
    bf = ml_dtypes.bfloat16
    B, S, HID = q.shape
    D = NH * DK
    n_hg = n_cores // B
    HTa = HID // P + (1 if aug else 0)

    def with_aug(xT, bias_row):
        pad = np.zeros((P, xT.shape[1]), xT.dtype)
        pad[0, :] = bias_row
        return np.concatenate([xT, pad], axis=0)

    def pack_w(w):
        # [HTa*P, D] -> [P, HTa*D] with chunk i in columns i*D..(i+1)*D
        return np.ascontiguousarray(
            w.reshape(HTa, P, D).transpose(1, 0, 2).reshape(P, HTa * D))

    per_batch = {}
    for b in range(B):
        qT = np.ascontiguousarray(q[b].T).astype(bf)
        kT = np.ascontiguousarray(k[b].T).astype(bf)
        vT = np.ascontiguousarray(v[b].T).astype(bf)
        if aug:
            one = np.ones((S,), np.float32).astype(bf)
            qT, kT, vT = with_aug(qT, one), with_aug(kT, one), with_aug(vT, one)
        per_batch[b] = (qT, kT, vT,
                        np.ascontiguousarray(mask[b, 0].T != 0).astype(bf))

    sel = np.zeros((2, P), np.float32)
    sel[0, 0:64] = 1.0
    sel[1, 64:128] = 1.0

    in_maps = []
    for core in range(n_cores):
        b, hg = divmod(core, n_hg)
        hsl = slice(hg * D, (hg + 1) * D)
        wq = Wq[:, hsl].astype(bf)
        wk = Wk[:, hsl].astype(bf)
        wv = Wv[:, hsl].astype(bf)
        if aug:
            wq = with_aug(wq, bq[hsl].astype(bf))
            wk = with_aug(wk, bk[hsl].astype(bf))
            wv = with_aug(wv, bv[hsl].astype(bf))
        qT, kT, vT, mT = per_batch[b]
        in_maps.append(dict(
            qT=qT, kT=kT, vT=vT, maskT=mT,
            wq=pack_w(wq), wk=pack_w(wk), wv=pack_w(wv),
            wo=np.ascontiguousarray(Wo[hsl, :]).astype(bf),
            sel=sel,
        ))
    return in_maps


def combine_outputs(results, B, S, HID, bo, n_cores=8):
    n_hg = n_cores // B
    out = np.zeros((B, S, HID), np.float32)
    for core in range(n_cores):
        b = core // n_hg
        out[b] += results[core]["y"].T.astype(np.float32)
    return out + bo.astype(np.float32)


def run_mha(q, k, v, mask, Wq, bq, Wk, bk, Wv, bv, Wo, bo, trace=False):
    from concourse.bass_utils import run_bass_kernel_spmd

    B, S, HID = q.shape
    n_cores = 8
    aug = bool(np.any(bq) or np.any(bk) or np.any(bv))
    key = (S, HID, aug)
    if key not in _PROGRAM_CACHE:
        _PROGRAM_CACHE[key] = build_mha_program(S=S, HID=HID, aug=aug)
    nc = _PROGRAM_CACHE[key]
    in_maps = make_in_maps(q, k, v, mask, Wq, bq, Wk, bk, Wv, bv, Wo,
                           n_cores=n_cores, aug=aug)
    res = run_bass_kernel_spmd(nc, in_maps, list(range(n_cores)), trace=trace)
    out = combine_outputs(res.results, B, S, HID, bo, n_cores=n_cores)
    return out, res


def kernel(q, k, v, mask, Wq, bq, Wk, bk, Wv, bv, Wo, bo):
    q = np.asarray(q, np.float32)
    k = np.asarray(k, np.float32)
    v = np.asarray(v, np.float32)
    mask = np.asarray(mask)
    out, _ = run_mha(q, k, v, mask,
                     np.asarray(Wq, np.float32), np.asarray(bq, np.float32),
                     np.asarray(Wk, np.float32), np.asarray(bk, np.float32),
                     np.asarray(Wv, np.float32), np.asarray(bv, np.float32),
                     np.asarray(Wo, np.float32), np.asarray(bo, np.float32))
    return out
